# revision 54
# baseline (speedup 1.0000x reference)
"""Trainium2 Bass kernel for C = triu(triu(A) @ triu(B)), N=4096, fp32.

Math: with host-side triu masking of A and B, the product is upper-triangular
automatically; for output element (r, c) only k in [r, c] contributes.

Sharding (8 cores, SPMD, one NEFF): a 4x2 grid.
  - Rows: 4 row-groups, cyclic mod 4 at 128-row tile granularity. Core with
    row-group r owns row-tiles {4t + r : t = 0..7}.
  - Columns: 2 column-groups by 64-wide strip parity: core with parity h owns
    64-col strips {2u + h}. A vsuper v = 0..3 packs the core's 8 strips
    {16v + 2j + h : j = 0..7} into 512 contiguous columns. At 64-strip
    granularity both parities have IDENTICAL sparsity structure:
    strip j is live at k-tile k iff j >= j0(k,v) = max(0, k - 8v), and output
    slot t needs strips j >= jc0(v,t) = max(0, 4t - 8v) -- no SPMD union waste.

Per-core: 109056 matmul rows (45.4us at 2.4 GHz) and ~13.4 MB A+B in +
2.36 MB C out (bf16).

Schedule: A + B are fully SBUF-resident; every input DMA is issued eagerly at
program start on ONE queue (sync) in exact first-need order — a single queue
sustains ~330-380 B/ns while two concurrently-active queues halve each other.
Work order puts high-slot-count k first so the PE is never starved waiting on
low-work-per-byte tiles:
  P1: v3 k=[8..31, 0..7]  (8 psum banks)
  P2: v2 k=[8..23, 0..7]  (6 banks)
  P3: v1 k=[8..15, 0..7]  (4 banks)
  P4: v0 k=[0..7]         (2 banks; overlaps v1's drain+store chain)
Slots t>=2 stop at the hi-part end and drain during the lo block; t0/t1 stop
at phase end. The PSUM pool's ring binding is arranged (via allocation order)
so first-needed psums bind earliest-freed banks. ~11 warmup matmuls on zeroed
SBUF keep the PE continuously busy from preamble end until the first chunks
land, so the HAM clock gate + p-state ramp complete without reset. PSUM->SBUF
drains alternate Vector/Scalar; C stores ride the gpsimd queue except the
final tiny store, which uses the by-then-idle sync queue.
"""

import sys

for _p in ("/opt/trn_rl_repo", "/root/.axon_site/_ro/trn_rl_repo"):
    if _p not in sys.path:
        sys.path.insert(0, _p)

import numpy as np

N = 4096
P = 128
NCORES = 8
KT = N // P  # 32 k-tiles
SW = 512  # psum width (8 strips of 64)
N_WARM = 8  # keeps the PE continuously busy from preamble end (~7us) until
# the first B/A chunks have landed (~10.4us), so the HAM clock gate and
# p-state ramp complete without a reset

NSLOTS = {0: 2, 1: 4, 2: 6, 3: 8}
KMAX = {v: 8 * v + 7 for v in range(4)}


def _j0(k, v):
    return max(0, k - 8 * v)


def _wB(k, v):  # B block width (cols) at (v, k)
    return 64 * (8 - _j0(k, v))


def _jc0(v, t):
    return max(0, 4 * t - 8 * v)


def _wC(v, t):  # C block width (cols) for slot (v, t)
    return 64 * (8 - _jc0(v, t))


# --- A pack: k-major, slots t <= k//4, lhsT tiles [p=k-rows, 128 m-cols] ---
def _aoff(k):
    return 128 * sum(kk // 4 + 1 for kk in range(k))


A_COLS = _aoff(KT)  # 18432

# A chunks (aq2 split so the k=8 working set lands sooner)
A_CHUNKS = {
    "aq2x": (_aoff(8), _aoff(10)),
    "aq2y": (_aoff(10), _aoff(12)),
    "aq3": (_aoff(12), _aoff(16)),
    "aq4": (_aoff(16), _aoff(20)),
    "aq5": (_aoff(20), _aoff(24)),
    "aq6": (_aoff(24), _aoff(28)),
    "aq7": (_aoff(28), _aoff(32)),
    "aq01": (_aoff(0), _aoff(8)),
}


def _a_chunk_name(k):
    q = k // 4
    if q < 2:
        return "aq01"
    if q == 2:
        return "aq2x" if k < 10 else "aq2y"
    return f"aq{q}"

# --- execution k-orders per phase: high-slot-count k first, then k=0..7.
# Slots t>=2 only run in the hi part (4t >= 8), so they stop and drain at
# hi-part end, overlapping the lo part; t0/t1 (and v0) stop at phase end.
P1_KS = list(range(8, 32)) + list(range(0, 8))
P2_KS = list(range(8, 24)) + list(range(0, 8))
P3_KS = list(range(8, 16)) + list(range(0, 8))

# --- B chunk stream: (name, [(v, k), ...]) in exact consumption order ---
def _b_chunks():
    ch = []
    ch.append(("b3_a1", [(3, k) for k in range(8, 10)]))
    ch.append(("b3_a2", [(3, k) for k in range(10, 12)]))
    ch.append(("b3_b1", [(3, k) for k in range(12, 14)]))
    ch.append(("b3_b2", [(3, k) for k in range(14, 16)]))
    ch.append(("b3_c", [(3, k) for k in range(16, 20)]))
    ch.append(("b3_d", [(3, k) for k in range(20, 24)]))
    ch.append(("b3_e", [(3, k) for k in range(24, 28)]))
    ch.append(("b3_f", [(3, k) for k in range(28, 32)]))
    ch.append(("b3_lo", [(3, k) for k in range(0, 8)]))
    ch.append(("b2_a", [(2, k) for k in range(8, 16)]))
    ch.append(("b2_b", [(2, k) for k in range(16, 24)]))
    ch.append(("b2_lo", [(2, k) for k in range(0, 8)]))
    ch.append(("b1", [(1, k) for k in range(8, 16)] + [(1, k) for k in range(0, 8)]))
    ch.append(("b0", [(0, k) for k in range(0, 8)]))
    return ch


B_CHUNKS = _b_chunks()

# chunk column layout within the single B DRAM tensor
B_CHUNK_BASE = {}
B_BLOCK_OFF = {}  # (v,k) -> (chunk_name, col offset within chunk, width)
_b = 0
for _name, _blocks in B_CHUNKS:
    B_CHUNK_BASE[_name] = _b
    _o = 0
    for _v, _k in _blocks:
        B_BLOCK_OFF[(_v, _k)] = (_name, _o, _wB(_k, _v))
        _o += _wB(_k, _v)
    _b += _o
B_COLS = _b  # 33792

# Single-queue DMA issue order (= exact first-need order). Two concurrently
# active queues halve each other's rate, so everything rides the sync queue.
STREAM = [
    ("A", "aq2x"),
    ("B", "b3_a1"),
    ("A", "aq2y"),
    ("B", "b3_a2"),
    ("A", "aq3"),
    ("B", "b3_b1"),
    ("B", "b3_b2"),
    ("A", "aq4"),
    ("B", "b3_c"),
    ("A", "aq5"),
    ("B", "b3_d"),
    ("A", "aq6"),
    ("B", "b3_e"),
    ("A", "aq7"),
    ("B", "b3_f"),
    ("A", "aq01"),
    ("B", "b3_lo"),
    ("B", "b2_a"),
    ("B", "b2_b"),
    ("B", "b2_lo"),
    ("B", "b1"),
    ("B", "b0"),
]

# --- C layout: blocks in drain-batch order ---
CBLOCKS = (
    [(3, t) for t in range(2, 8)]
    + [(3, 0), (3, 1)]
    + [(2, t) for t in range(2, 6)]
    + [(2, 0), (2, 1)]
    + [(1, 2), (1, 3)]
    + [(1, 0), (1, 1)]
    + [(0, 0), (0, 1)]
)
CBASE = {}
_c = 0
for _v, _t in CBLOCKS:
    CBASE[(_v, _t)] = _c
    _c += _wC(_v, _t)
C_COLS = _c  # 9216



_cache = {}


def _build():
    import concourse.bacc as bacc
    import concourse.mybir as mybir
    import concourse.tile as tile

    D = mybir.dt.bfloat16
    F32 = mybir.dt.float32
    Copy = mybir.ActivationFunctionType.Copy

    nc = bacc.Bacc(None, target_bir_lowering=False)
    AT = nc.dram_tensor("AT", [P, A_COLS], D, kind="ExternalInput")
    Bm = nc.dram_tensor("B", [P, B_COLS], D, kind="ExternalInput")
    Cm = nc.dram_tensor("C", [P, C_COLS], D, kind="ExternalOutput")

    with tile.TileContext(nc) as tc:
        with (
            tc.tile_pool(name="w", bufs=1) as wpool,
            tc.tile_pool(name="a", bufs=1) as apool,
            tc.tile_pool(name="b", bufs=1) as bpool,
            tc.tile_pool(name="o", bufs=1) as opool,
            tc.tile_pool(name="ps", bufs=8, space="PSUM") as pspool,
        ):
            # --- PE warmup during preamble / first DMA window ---
            warm = wpool.tile([P, SW], D, tag="wm", name="warm")
            nc.vector.memset(warm[:], 0)
            wps = pspool.tile([P, SW], F32, tag="ps", name="ps")
            for _ in range(N_WARM):
                nc.tensor.matmul(
                    wps[:], warm[:, :P], warm[:], start=True, stop=True
                )

            # --- eager input DMAs, one queue, exact need order ---
            a_tiles = {}
            b_tiles = {}
            b_chunk_map = dict(B_CHUNKS)
            for kind, name in STREAM:
                if kind == "A":
                    c0, c1 = A_CHUNKS[name]
                    at = apool.tile([P, c1 - c0], D, tag=name, name="at")
                    nc.sync.dma_start(at[:], AT[:, c0:c1])
                    a_tiles[name] = (at, c0)
                else:
                    base = B_CHUNK_BASE[name]
                    w = sum(_wB(k, v) for v, k in b_chunk_map[name])
                    bt = bpool.tile([P, w], D, tag=name, name="bt")
                    nc.sync.dma_start(bt[:], Bm[:, base : base + w])
                    b_tiles[name] = bt

            def a_sl(k, t):
                at, c0 = a_tiles[_a_chunk_name(k)]
                off = _aoff(k) - c0 + 128 * t
                return at[:, off : off + 128]

            def b_sl(v, k):
                name, off, w = B_BLOCK_OFF[(v, k)]
                return b_tiles[name][:, off : off + w]

            # --- psum allocation helper (ring order is load-bearing) ---
            def alloc_ps():
                return pspool.tile([P, SW], F32, tag="ps", name="ps")

            def mm(ps, v, k, t, first_k, last_k):
                w0 = 64 * _j0(k, v)
                nc.tensor.matmul(
                    ps[:, w0:SW],
                    a_sl(k, t),
                    b_sl(v, k),
                    start=(k == first_k),
                    stop=(k == last_k),
                )

            drain_eng = [0]  # alternate vector/scalar

            def drain_and_store(batch_name, blocks, psums, n_dmas):
                cols = sum(_wC(v, t) for v, t in blocks)
                ot = opool.tile([P, cols], D, tag=batch_name, name="ot")
                l0 = 0
                for v, t in blocks:
                    w0 = 64 * _jc0(v, t)
                    cw = _wC(v, t)
                    src = psums[(v, t)][:, w0:SW]
                    if drain_eng[0] % 2 == 0:
                        nc.vector.tensor_copy(ot[:, l0 : l0 + cw], src)
                    else:
                        nc.scalar.activation(ot[:, l0 : l0 + cw], src, Copy)
                    drain_eng[0] += 1
                    l0 += cw
                base = CBASE[blocks[0]]
                splits = [cols * i // n_dmas for i in range(n_dmas + 1)]
                for s0, s1 in zip(splits, splits[1:]):
                    nc.gpsimd.dma_start(
                        Cm[:, base + s0 : base + s1], ot[:, s0:s1]
                    )

            # ============ P1: v3, k = [8..31, 0..7] ============
            ps3 = {}
            for t in range(8):
                ps3[(3, t)] = alloc_ps()
            for k in P1_KS:
                for t in range(min(k // 4, 7) + 1):
                    first_k = 4 * t if 4 * t >= 8 else 8
                    last_k = 7 if t <= 1 else 31
                    mm(ps3[(3, t)], 3, k, t, first_k, last_k)
                if k == 31:  # t2..t7 complete: drain during the lo block
                    drain_and_store("c3hi", [(3, t) for t in range(2, 8)], ps3, 2)
            drain_and_store("c3lo", [(3, 0), (3, 1)], ps3, 1)

            # ============ P2: v2, k = [8..23, 0..7] ============
            ps2 = {}
            # alloc order binds ring bufs: t4/t5 (first used at k16/k20)
            # absorb v3's late-freed t0/t1 banks, so t0..t3 (needed at
            # k8-k12) bind banks freed at v3's k=31 -- no boundary stall.
            for t in (4, 5, 0, 1, 2, 3):
                ps2[(2, t)] = alloc_ps()
            for k in P2_KS:
                for t in range(min(k // 4, 5) + 1):
                    first_k = 4 * t if 4 * t >= 8 else 8
                    last_k = 7 if t <= 1 else 23
                    mm(ps2[(2, t)], 2, k, t, first_k, last_k)
                if k == 23:  # t2..t5 complete: drain during the lo block
                    drain_and_store("c2hi", [(2, t) for t in range(2, 6)], ps2, 1)
            drain_and_store("c2lo", [(2, 0), (2, 1)], ps2, 1)

            # ============ P3: v1, k = [8..15, 0..7] ============
            ps1 = {}
            for t in range(4):
                ps1[(1, t)] = alloc_ps()
            for k in P3_KS:
                ts = list(range(min(k // 4, 3) + 1))
                if k == 7:
                    ts = [1, 0]  # t1 stops first so its drain starts earlier
                for t in ts:
                    first_k = 4 * t if 4 * t >= 8 else 8
                    last_k = 7 if t <= 1 else 15
                    mm(ps1[(1, t)], 1, k, t, first_k, last_k)
                if k == 15:  # t2, t3 complete: drain during the lo block
                    drain_and_store("c1hi", [(1, 2), (1, 3)], ps1, 1)

            # ============ P4: v0, k = 0..7 (2 banks, long free) ============
            # v0's matmuls run on the PE while v1's t0/t1 drain+store chains
            # execute on Vector/Scalar and the sync/scalar DMA queues.
            ps0 = {(0, t): alloc_ps() for t in range(2)}
            for k in range(8):
                for t in range(min(k // 4, 1) + 1):
                    mm(ps0[(0, t)], 0, k, t, 4 * t, 7)

            # v1 t0/t1: per-slot copy+store, each store gated on its own copy
            ot1 = opool.tile([P, _wC(1, 1)], D, tag="c1t1", name="ot1")
            nc.scalar.activation(ot1[:], ps1[(1, 1)][:, 0:SW], Copy)
            nc.sync.dma_start(
                Cm[:, CBASE[(1, 1)] : CBASE[(1, 1)] + _wC(1, 1)], ot1[:]
            )
            ot0 = opool.tile([P, _wC(1, 0)], D, tag="c1t0", name="ot0")
            nc.vector.tensor_copy(ot0[:], ps1[(1, 0)][:, 0:SW])
            nc.scalar.dma_start(
                Cm[:, CBASE[(1, 0)] : CBASE[(1, 0)] + _wC(1, 0)], ot0[:]
            )
            # final: v0 drain + one small store on the (idle) sync queue
            otf = opool.tile([P, _wC(0, 0) + _wC(0, 1)], D, tag="c0", name="otf")
            nc.vector.tensor_copy(otf[:, : _wC(0, 0)], ps0[(0, 0)][:, 0:SW])
            nc.scalar.activation(
                otf[:, _wC(0, 0) :], ps0[(0, 1)][:, 64 * _jc0(0, 1) : SW], Copy
            )
            base = CBASE[(0, 0)]
            nc.sync.dma_start(
                Cm[:, base : base + _wC(0, 0) + _wC(0, 1)], otf[:]
            )

    nc.compile()
    return nc


def _get_nc():
    if "nc" not in _cache:
        _cache["nc"] = _build()
    return _cache["nc"]


def _make_in_maps(A, B):
    import ml_dtypes

    bf16 = np.dtype(ml_dtypes.bfloat16)
    A = np.asarray(A, dtype=np.float32)
    B = np.asarray(B, dtype=np.float32)
    Au = np.triu(A).astype(bf16)
    Bu = np.triu(B).astype(bf16)

    # A packs per row-group r: [p, k-major slots], lhsT layout
    a_packs = []
    for r in range(4):
        ATr = np.zeros((P, A_COLS), dtype=bf16)
        for k in range(KT):
            base = _aoff(k)
            for t in range(k // 4 + 1):
                m = 4 * t + r
                ATr[:, base + 128 * t : base + 128 * (t + 1)] = Au[
                    128 * m : 128 * m + 128, 128 * k : 128 * k + 128
                ].T
        a_packs.append(ATr)

    # B packs per column parity h: chunk-stream layout, 64-wide strips
    b_packs = []
    for h in range(2):
        Bh = np.zeros((P, B_COLS), dtype=bf16)
        for name, blocks in B_CHUNKS:
            base = B_CHUNK_BASE[name]
            for v, k in blocks:
                _, off, w = B_BLOCK_OFF[(v, k)]
                for i, j in enumerate(range(_j0(k, v), 8)):
                    n64 = 16 * v + 2 * j + h
                    Bh[:, base + off + 64 * i : base + off + 64 * (i + 1)] = (
                        Bu[128 * k : 128 * k + 128, 64 * n64 : 64 * n64 + 64]
                    )
        b_packs.append(Bh)

    in_maps = []
    for jcore in range(NCORES):
        r, h = jcore % 4, jcore // 4
        in_maps.append({"AT": a_packs[r], "B": b_packs[h]})
    return in_maps


def kernel(A, B):
    from concourse.bass_utils import run_bass_kernel_spmd

    in_maps = _make_in_maps(A, B)
    nc = _get_nc()
    res = run_bass_kernel_spmd(nc, in_maps, core_ids=list(range(NCORES)))

    C = np.zeros((N, N), dtype=np.float32)
    for jcore in range(NCORES):
        r, h = jcore % 4, jcore // 4
        Cj = np.asarray(res.results[jcore]["C"]).astype(np.float32)
        for v, t in CBLOCKS:
            m = 4 * t + r
            cb = CBASE[(v, t)]
            for i, j in enumerate(range(_jc0(v, t), 8)):
                n64 = 16 * v + 2 * j + h
                C[128 * m : 128 * m + 128, 64 * n64 : 64 * n64 + 64] = Cj[
                    :, cb + 64 * i : cb + 64 * (i + 1)
                ]
    return C


# revision 55
# speedup vs baseline: 1.0429x; 1.0429x over previous
"""Trainium2 Bass kernel for C = triu(triu(A) @ triu(B)), N=4096, fp32.

Math: with host-side triu masking of A and B, the product is upper-triangular
automatically; for output element (r, c) only k in [r, c] contributes.

Sharding (8 cores, SPMD, one NEFF): a 4x2 grid.
  - Rows: 4 row-groups, cyclic mod 4 at 128-row tile granularity. Core with
    row-group r owns row-tiles {4t + r : t = 0..7}.
  - Columns: 2 column-groups by 64-wide strip parity: core with parity h owns
    64-col strips {2u + h}. A vsuper v = 0..3 packs the core's 8 strips
    {16v + 2j + h : j = 0..7} into 512 contiguous columns. At 64-strip
    granularity both parities have IDENTICAL sparsity structure:
    strip j is live at k-tile k iff j >= j0(k,v) = max(0, k - 8v), and output
    slot t needs strips j >= jc0(v,t) = max(0, 4t - 8v) -- no SPMD union waste.

Per-core: 109056 matmul rows (45.4us at 2.4 GHz) and ~13.4 MB A+B in +
2.36 MB C out (bf16).

Schedule: A + B are fully SBUF-resident; every input DMA is issued eagerly at
program start on ONE queue (sync) in exact first-need order — a single queue
sustains ~330-380 B/ns while two concurrently-active queues halve each other.
Work order puts high-slot-count k first so the PE is never starved waiting on
low-work-per-byte tiles:
  P1: v3 k=[8..31, 0..7]  (8 psum banks)
  P2: v2 k=[8..23, 0..7]  (6 banks)
  P3: v1 k=[8..15, 0..7]  (4 banks)
  P4: v0 k=[0..7]         (2 banks; overlaps v1's drain+store chain)
Slots t>=2 stop at the hi-part end and drain during the lo block; t0/t1 stop
at phase end. The PSUM pool's ring binding is arranged (via allocation order)
so first-needed psums bind earliest-freed banks. ~11 warmup matmuls on zeroed
SBUF keep the PE continuously busy from preamble end until the first chunks
land, so the HAM clock gate + p-state ramp complete without reset. PSUM->SBUF
drains alternate Vector/Scalar; C stores ride the gpsimd queue except the
final tiny store, which uses the by-then-idle sync queue.
"""

import sys

for _p in ("/opt/trn_rl_repo", "/root/.axon_site/_ro/trn_rl_repo"):
    if _p not in sys.path:
        sys.path.insert(0, _p)

import numpy as np

N = 4096
P = 128
NCORES = 8
KT = N // P  # 32 k-tiles
SW = 512  # psum width (8 strips of 64)
N_WARM = 10  # keeps the PE continuously busy from preamble end (~7us) until
# the first B/A chunks have landed (~11.3us), so the HAM clock gate and
# p-state ramp complete without a reset

NSLOTS = {0: 2, 1: 4, 2: 6, 3: 8}
KMAX = {v: 8 * v + 7 for v in range(4)}


def _j0(k, v):
    return max(0, k - 8 * v)


def _wB(k, v):  # B block width (cols) at (v, k)
    return 64 * (8 - _j0(k, v))


def _jc0(v, t):
    return max(0, 4 * t - 8 * v)


def _wC(v, t):  # C block width (cols) for slot (v, t)
    return 64 * (8 - _jc0(v, t))


# --- A pack: k-major, slots t <= k//4, lhsT tiles [p=k-rows, 128 m-cols] ---
def _aoff(k):
    return 128 * sum(kk // 4 + 1 for kk in range(k))


A_COLS = _aoff(KT)  # 18432

# A chunks (aq2 split so the k=8 working set lands sooner)
A_CHUNKS = {
    "aq2x": (_aoff(8), _aoff(10)),
    "aq2y": (_aoff(10), _aoff(12)),
    "aq3": (_aoff(12), _aoff(16)),
    "aq4": (_aoff(16), _aoff(20)),
    "aq5": (_aoff(20), _aoff(24)),
    "aq6": (_aoff(24), _aoff(28)),
    "aq7": (_aoff(28), _aoff(32)),
    "aq01": (_aoff(0), _aoff(8)),
}


def _a_chunk_name(k):
    q = k // 4
    if q < 2:
        return "aq01"
    if q == 2:
        return "aq2x" if k < 10 else "aq2y"
    return f"aq{q}"

# --- execution k-orders per phase: high-slot-count k first, then k=0..7.
# Slots t>=2 only run in the hi part (4t >= 8), so they stop and drain at
# hi-part end, overlapping the lo part; t0/t1 (and v0) stop at phase end.
P1_KS = list(range(8, 32)) + list(range(0, 8))
P2_KS = list(range(8, 24)) + list(range(0, 8))
P3_KS = list(range(8, 16)) + list(range(0, 8))

# --- B chunk stream: (name, [(v, k), ...]) in exact consumption order ---
def _b_chunks():
    ch = []
    ch.append(("b3_a", [(3, k) for k in range(8, 12)]))
    ch.append(("b3_b1", [(3, k) for k in range(12, 14)]))
    ch.append(("b3_b2", [(3, k) for k in range(14, 16)]))
    ch.append(("b3_c", [(3, k) for k in range(16, 20)]))
    ch.append(("b3_d", [(3, k) for k in range(20, 24)]))
    ch.append(("b3_e", [(3, k) for k in range(24, 28)]))
    ch.append(("b3_f", [(3, k) for k in range(28, 32)]))
    ch.append(("b3_lo", [(3, k) for k in range(0, 8)]))
    ch.append(("b2_a", [(2, k) for k in range(8, 16)]))
    ch.append(("b2_b", [(2, k) for k in range(16, 24)]))
    ch.append(("b2_lo", [(2, k) for k in range(0, 8)]))
    ch.append(("b1", [(1, k) for k in range(8, 16)] + [(1, k) for k in range(0, 8)]))
    ch.append(("b0", [(0, k) for k in range(0, 8)]))
    return ch


B_CHUNKS = _b_chunks()

# chunk column layout within the single B DRAM tensor
B_CHUNK_BASE = {}
B_BLOCK_OFF = {}  # (v,k) -> (chunk_name, col offset within chunk, width)
_b = 0
for _name, _blocks in B_CHUNKS:
    B_CHUNK_BASE[_name] = _b
    _o = 0
    for _v, _k in _blocks:
        B_BLOCK_OFF[(_v, _k)] = (_name, _o, _wB(_k, _v))
        _o += _wB(_k, _v)
    _b += _o
B_COLS = _b  # 33792

# Single-queue DMA issue order (= exact first-need order). Two concurrently
# active queues halve each other's rate, so everything rides the sync queue.
STREAM = [
    ("A", "aq2x"),
    ("B", "b3_a"),
    ("A", "aq2y"),
    ("A", "aq3"),
    ("B", "b3_b1"),
    ("B", "b3_b2"),
    ("A", "aq4"),
    ("B", "b3_c"),
    ("A", "aq5"),
    ("B", "b3_d"),
    ("A", "aq6"),
    ("B", "b3_e"),
    ("A", "aq7"),
    ("B", "b3_f"),
    ("A", "aq01"),
    ("B", "b3_lo"),
    ("B", "b2_a"),
    ("B", "b2_b"),
    ("B", "b2_lo"),
    ("B", "b1"),
    ("B", "b0"),
]

# --- C layout: blocks in drain-batch order ---
CBLOCKS = (
    [(3, t) for t in range(2, 8)]
    + [(3, 0), (3, 1)]
    + [(2, t) for t in range(2, 6)]
    + [(2, 0), (2, 1)]
    + [(1, 2), (1, 3)]
    + [(1, 0), (1, 1)]
    + [(0, 0), (0, 1)]
)
CBASE = {}
_c = 0
for _v, _t in CBLOCKS:
    CBASE[(_v, _t)] = _c
    _c += _wC(_v, _t)
C_COLS = _c  # 9216



_cache = {}


def _build():
    import concourse.bacc as bacc
    import concourse.mybir as mybir
    import concourse.tile as tile

    D = mybir.dt.bfloat16
    F32 = mybir.dt.float32
    Copy = mybir.ActivationFunctionType.Copy

    nc = bacc.Bacc(None, target_bir_lowering=False)
    AT = nc.dram_tensor("AT", [P, A_COLS], D, kind="ExternalInput")
    Bm = nc.dram_tensor("B", [P, B_COLS], D, kind="ExternalInput")
    Cm = nc.dram_tensor("C", [P, C_COLS], D, kind="ExternalOutput")

    with tile.TileContext(nc) as tc:
        with (
            tc.tile_pool(name="w", bufs=1) as wpool,
            tc.tile_pool(name="a", bufs=1) as apool,
            tc.tile_pool(name="b", bufs=1) as bpool,
            tc.tile_pool(name="o", bufs=1) as opool,
            tc.tile_pool(name="ps", bufs=8, space="PSUM") as pspool,
        ):
            # --- PE warmup during preamble / first DMA window ---
            warm = wpool.tile([P, SW], D, tag="wm", name="warm")
            nc.vector.memset(warm[:], 0)
            wps = pspool.tile([P, SW], F32, tag="ps", name="ps")
            for _ in range(N_WARM):
                nc.tensor.matmul(
                    wps[:], warm[:, :P], warm[:], start=True, stop=True
                )

            # --- eager input DMAs, one queue, exact need order ---
            a_tiles = {}
            b_tiles = {}
            b_chunk_map = dict(B_CHUNKS)
            for kind, name in STREAM:
                if kind == "A":
                    c0, c1 = A_CHUNKS[name]
                    at = apool.tile([P, c1 - c0], D, tag=name, name="at")
                    nc.sync.dma_start(at[:], AT[:, c0:c1])
                    a_tiles[name] = (at, c0)
                else:
                    base = B_CHUNK_BASE[name]
                    w = sum(_wB(k, v) for v, k in b_chunk_map[name])
                    bt = bpool.tile([P, w], D, tag=name, name="bt")
                    nc.sync.dma_start(bt[:], Bm[:, base : base + w])
                    b_tiles[name] = bt

            def a_sl(k, t):
                at, c0 = a_tiles[_a_chunk_name(k)]
                off = _aoff(k) - c0 + 128 * t
                return at[:, off : off + 128]

            def b_sl(v, k):
                name, off, w = B_BLOCK_OFF[(v, k)]
                return b_tiles[name][:, off : off + w]

            # --- psum allocation helper (ring order is load-bearing) ---
            def alloc_ps():
                return pspool.tile([P, SW], F32, tag="ps", name="ps")

            def mm(ps, v, k, t, first_k, last_k):
                w0 = 64 * _j0(k, v)
                nc.tensor.matmul(
                    ps[:, w0:SW],
                    a_sl(k, t),
                    b_sl(v, k),
                    start=(k == first_k),
                    stop=(k == last_k),
                )

            drain_eng = [0]  # alternate vector/scalar

            def drain_and_store(batch_name, blocks, psums, n_dmas):
                cols = sum(_wC(v, t) for v, t in blocks)
                ot = opool.tile([P, cols], D, tag=batch_name, name="ot")
                l0 = 0
                for v, t in blocks:
                    w0 = 64 * _jc0(v, t)
                    cw = _wC(v, t)
                    src = psums[(v, t)][:, w0:SW]
                    if drain_eng[0] % 2 == 0:
                        nc.vector.tensor_copy(ot[:, l0 : l0 + cw], src)
                    else:
                        nc.scalar.activation(ot[:, l0 : l0 + cw], src, Copy)
                    drain_eng[0] += 1
                    l0 += cw
                base = CBASE[blocks[0]]
                splits = [cols * i // n_dmas for i in range(n_dmas + 1)]
                for s0, s1 in zip(splits, splits[1:]):
                    nc.gpsimd.dma_start(
                        Cm[:, base + s0 : base + s1], ot[:, s0:s1]
                    )

            # ============ P1: v3, k = [8..31, 0..7] ============
            ps3 = {}
            for t in range(8):
                ps3[(3, t)] = alloc_ps()
            for k in P1_KS:
                for t in range(min(k // 4, 7) + 1):
                    first_k = 4 * t if 4 * t >= 8 else 8
                    last_k = 7 if t <= 1 else 31
                    mm(ps3[(3, t)], 3, k, t, first_k, last_k)
                if k == 31:  # t2..t7 complete: drain during the lo block
                    drain_and_store("c3hi", [(3, t) for t in range(2, 8)], ps3, 2)
            drain_and_store("c3lo", [(3, 0), (3, 1)], ps3, 1)

            # ============ P2: v2, k = [8..23, 0..7] ============
            ps2 = {}
            # alloc order binds ring bufs: t4/t5 (first used at k16/k20)
            # absorb v3's late-freed t0/t1 banks, so t0..t3 (needed at
            # k8-k12) bind banks freed at v3's k=31 -- no boundary stall.
            for t in (4, 5, 0, 1, 2, 3):
                ps2[(2, t)] = alloc_ps()
            for k in P2_KS:
                for t in range(min(k // 4, 5) + 1):
                    first_k = 4 * t if 4 * t >= 8 else 8
                    last_k = 7 if t <= 1 else 23
                    mm(ps2[(2, t)], 2, k, t, first_k, last_k)
                if k == 23:  # t2..t5 complete: drain during the lo block
                    drain_and_store("c2hi", [(2, t) for t in range(2, 6)], ps2, 1)
            drain_and_store("c2lo", [(2, 0), (2, 1)], ps2, 1)

            # ============ P3: v1, k = [8..15, 0..7] ============
            ps1 = {}
            for t in range(4):
                ps1[(1, t)] = alloc_ps()
            for k in P3_KS:
                ts = list(range(min(k // 4, 3) + 1))
                if k == 7:
                    ts = [1, 0]  # t1 stops first so its drain starts earlier
                for t in ts:
                    first_k = 4 * t if 4 * t >= 8 else 8
                    last_k = 7 if t <= 1 else 15
                    mm(ps1[(1, t)], 1, k, t, first_k, last_k)
                if k == 15:  # t2, t3 complete: drain during the lo block
                    drain_and_store("c1hi", [(1, 2), (1, 3)], ps1, 1)

            # ============ P4: v0, k = 0..7 (2 banks, long free) ============
            # v0's matmuls run on the PE while v1's t0/t1 drain+store chains
            # execute on Vector/Scalar and the sync/scalar DMA queues.
            ps0 = {(0, t): alloc_ps() for t in range(2)}
            for k in range(8):
                for t in range(min(k // 4, 1) + 1):
                    mm(ps0[(0, t)], 0, k, t, 4 * t, 7)

            # v1 t0/t1: per-slot copy+store, each store gated on its own copy
            ot1 = opool.tile([P, _wC(1, 1)], D, tag="c1t1", name="ot1")
            nc.scalar.activation(ot1[:], ps1[(1, 1)][:, 0:SW], Copy)
            nc.sync.dma_start(
                Cm[:, CBASE[(1, 1)] : CBASE[(1, 1)] + _wC(1, 1)], ot1[:]
            )
            ot0 = opool.tile([P, _wC(1, 0)], D, tag="c1t0", name="ot0")
            nc.vector.tensor_copy(ot0[:], ps1[(1, 0)][:, 0:SW])
            nc.scalar.dma_start(
                Cm[:, CBASE[(1, 0)] : CBASE[(1, 0)] + _wC(1, 0)], ot0[:]
            )
            # final: v0 drain + one small store on the (idle) sync queue
            otf = opool.tile([P, _wC(0, 0) + _wC(0, 1)], D, tag="c0", name="otf")
            nc.vector.tensor_copy(otf[:, : _wC(0, 0)], ps0[(0, 0)][:, 0:SW])
            nc.scalar.activation(
                otf[:, _wC(0, 0) :], ps0[(0, 1)][:, 64 * _jc0(0, 1) : SW], Copy
            )
            base = CBASE[(0, 0)]
            nc.sync.dma_start(
                Cm[:, base : base + _wC(0, 0) + _wC(0, 1)], otf[:]
            )

    nc.compile()
    return nc


def _get_nc():
    if "nc" not in _cache:
        _cache["nc"] = _build()
    return _cache["nc"]


def _make_in_maps(A, B):
    import ml_dtypes

    bf16 = np.dtype(ml_dtypes.bfloat16)
    A = np.asarray(A, dtype=np.float32)
    B = np.asarray(B, dtype=np.float32)
    Au = np.triu(A).astype(bf16)
    Bu = np.triu(B).astype(bf16)

    # A packs per row-group r: [p, k-major slots], lhsT layout
    a_packs = []
    for r in range(4):
        ATr = np.zeros((P, A_COLS), dtype=bf16)
        for k in range(KT):
            base = _aoff(k)
            for t in range(k // 4 + 1):
                m = 4 * t + r
                ATr[:, base + 128 * t : base + 128 * (t + 1)] = Au[
                    128 * m : 128 * m + 128, 128 * k : 128 * k + 128
                ].T
        a_packs.append(ATr)

    # B packs per column parity h: chunk-stream layout, 64-wide strips
    b_packs = []
    for h in range(2):
        Bh = np.zeros((P, B_COLS), dtype=bf16)
        for name, blocks in B_CHUNKS:
            base = B_CHUNK_BASE[name]
            for v, k in blocks:
                _, off, w = B_BLOCK_OFF[(v, k)]
                for i, j in enumerate(range(_j0(k, v), 8)):
                    n64 = 16 * v + 2 * j + h
                    Bh[:, base + off + 64 * i : base + off + 64 * (i + 1)] = (
                        Bu[128 * k : 128 * k + 128, 64 * n64 : 64 * n64 + 64]
                    )
        b_packs.append(Bh)

    in_maps = []
    for jcore in range(NCORES):
        r, h = jcore % 4, jcore // 4
        in_maps.append({"AT": a_packs[r], "B": b_packs[h]})
    return in_maps


def kernel(A, B):
    from concourse.bass_utils import run_bass_kernel_spmd

    in_maps = _make_in_maps(A, B)
    nc = _get_nc()
    res = run_bass_kernel_spmd(nc, in_maps, core_ids=list(range(NCORES)))

    C = np.zeros((N, N), dtype=np.float32)
    for jcore in range(NCORES):
        r, h = jcore % 4, jcore // 4
        Cj = np.asarray(res.results[jcore]["C"]).astype(np.float32)
        for v, t in CBLOCKS:
            m = 4 * t + r
            cb = CBASE[(v, t)]
            for i, j in enumerate(range(_jc0(v, t), 8)):
                n64 = 16 * v + 2 * j + h
                C[128 * m : 128 * m + 128, 64 * n64 : 64 * n64 + 64] = Cj[
                    :, cb + 64 * i : cb + 64 * (i + 1)
                ]
    return C
